# revision 3
# baseline (speedup 1.0000x reference)
"""NT-Xent contrastive loss (forward) on 8 TRN2 NeuronCores via Bass/Tile.

v2: fp8(e4m3) DoubleRow matmuls + symmetric (half-matrix) evaluation.

Math: with h = concat(h_i, h_j) [N=8192, D=256], sim = (h @ h.T) / 0.5,
loss = mean_r( logsumexp_j(sim[r, j], j != r) - pos_r ).  pos is computed
exactly on the host (f64); the device produces per-row partial sums of
exp(sim - M) with a single global shift M = 161.

Symmetry: exp(sim[i,j] - M) == exp(sim[j,i] - M) for a global M, so each
128-row block only computes the cyclic column distances d = 0..32 (of 64
128-col blocks).  Row-sums over d = 0..32 come from ACT (exp + accum_out)
and DVE (Schraudolph bit-trick exp + fused tensor_tensor_reduce); the
remaining distances d = 33..63 for every row are recovered as COLUMN sums
of the d = 1..31 exp tiles, computed by the PE as ones-weighted matmuls
accumulating into a single PSUM bank laid out as [10, 512] f32.

Sharding: core c owns rows [1024c, 1024c + 1024).  Each core receives h.T
column-rotated by its row offset in fp8, pre-transposed to the DoubleRow
layout [128, 2, 5120] (only rotated columns [0, 5120) are ever touched:
row-block r uses columns [128r, 128r + 4224)).  One SPMD program serves
all 8 cores; the host reassembles row sums + column sums in f64, takes
logs, and averages.  Rows whose device sum is non-finite (never, for the
reference data) are recomputed exactly on the host.
"""

import os

import numpy as np
import ml_dtypes

DBG_NO_DR = os.environ.get("DBG_NO_DR", "") != ""
DBG_NO_COLSUM = os.environ.get("DBG_NO_COLSUM", "") != ""
DBG_NO_TTR = os.environ.get("DBG_NO_TTR", "") != ""

B = 4096
D = 256
N = 2 * B
NCORES = 8
RPC = N // NCORES     # 1024 rows per core
NRB = RPC // 128      # 8 row-blocks of 128 per core
M_DEFAULT = 161.0     # global logsumexp shift (rowmax(2 h.h) in [102, 240])
MASK_NEG = -1.0e9

SIMW = 33 * 128       # 4224: sim columns per row-block (distances 0..32)
CSW = 31 * 128        # 3968: colsum columns (distances 1..31)
RHSW = 7 * 128 + SIMW # 5120: rotated columns a core ever reads
HSPLIT = 4352         # first h.T tile: covers row-blocks 0 and 1 entirely
ACTW = 3072           # columns [0, 3072) of each row-block go to ACT
NSLOT = 6             # res slots per row-block (2 ACT + 3 DVE, 1 spare)

# Schraudolph fast-exp in bf16-bit space: exp(y) ~= bitcast_bf16(u16(A*y+B)).
EXP_A16 = float(2 ** 7 / np.log(2.0))
EXP_B16 = 16248.55

TRACE = False
LAST_RESULTS = None

_cache = {}


def _build():
    if "nc" in _cache:
        return _cache["nc"]

    import concourse.tile as tile
    import concourse.mybir as mybir
    from concourse import bacc

    f32 = mybir.dt.float32
    bf16 = mybir.dt.bfloat16
    f8 = mybir.dt.float8e4
    u16 = mybir.dt.uint16
    DR = mybir.MatmulPerfMode.DoubleRow
    ALU = mybir.AluOpType

    nc = bacc.Bacc("TRN2", target_bir_lowering=False, num_devices=NCORES)
    # ht's first 256 columns hold the DoubleRow identity and -240*I mask
    # planes so row 0's first matmuls ride the first DMA chunk.
    ht_dram = nc.dram_tensor("ht", [128, 2, 256 + RHSW], f8, kind="ExternalInput").ap()
    res_dram = nc.dram_tensor("res", [128, NRB * NSLOT], f32, kind="ExternalOutput").ap()
    csel_dram = nc.dram_tensor("csel", [128, 4, 4], bf16, kind="ExternalInput").ap()
    csum_dram = nc.dram_tensor("csum", [68, 512], f32, kind="ExternalOutput").ap()

    with tile.TileContext(nc) as tc:
        with (
            tc.tile_pool(name="hpool", bufs=1) as hpool,
            tc.tile_pool(name="small", bufs=1) as small,
            tc.tile_pool(name="exppool", bufs=2) as exppool,
            tc.tile_pool(name="tipool", bufs=4) as tipool,
            tc.tile_pool(name="psumA", bufs=2, space="PSUM") as psumA,
            tc.tile_pool(name="psumB", bufs=1, space="PSUM") as psumB,
            tc.tile_pool(name="psumC", bufs=1, space="PSUM") as psumC,
        ):
            # --- constants / warmup (overlap the big DMAs) ---

            ones_sb = small.tile([128, 512], bf16)
            nc.vector.memset(ones_sb, 1.0)
            zsel_sb = small.tile([128, 4], bf16)
            nc.vector.memset(zsel_sb, 0.0)
            csel_sb = small.tile([128, 4, 4], bf16)
            nc.gpsimd.dma_start(out=csel_sb, in_=csel_dram)
            wsrc = small.tile([128, 128], bf16)
            nc.vector.memset(wsrc, 0.0)
            res_sb = small.tile([128, NRB * NSLOT], f32)
            nc.vector.memset(res_sb, 0.0)
            csum_sb = small.tile([68, 512], f32)

            # ACT exp-table warm (one tiny activation loads the table).
            warm32 = small.tile([128, 1], f32)
            nc.vector.memset(warm32, 0.0)
            biasm = small.tile([128, 1], f32)
            nc.vector.memset(biasm, -M_DEFAULT)
            nc.scalar.activation(
                out=warm32, in_=warm32,
                func=mybir.ActivationFunctionType.Exp, bias=0.0, scale=0.0,
            )

            # h.T in fp8 DoubleRow layout, split so compute starts on the
            # first piece while the rest streams in.
            ht_tiles = []   # (rotated c0, c1, tile) -- tile col 0 = rot c0
            bounds = [-256, 512, 1536, 3072, RHSW]
            queues = [nc.sync, nc.sync, nc.gpsimd, nc.gpsimd]
            for bi in range(len(bounds) - 1):
                c0, c1 = bounds[bi], bounds[bi + 1]
                t = hpool.tile([128, 2, c1 - c0], f8, name=f"ht{c0}")
                queues[bi].dma_start(out=t, in_=ht_dram[:, :, c0 + 256:c1 + 256])
                ht_tiles.append((c0, c1, t))
            eye_sb = ht_tiles[0][2][:, :, 0:128]
            negd_sb = ht_tiles[0][2][:, :, 128:256]

            # PE HAM warm: ~32 dummy matmuls while the h.T DMA flies.
            wps = psumA.tile([128, 1536], f32, name="psA")
            for w in range(26):
                nc.tensor.matmul(
                    wps[:, (w % 3) * 512:(w % 3) * 512 + 128],
                    lhsT=wsrc, rhs=wsrc, start=True, stop=True,
                )

            # colsum accumulator: one PSUM bank; q-run r lands on partition
            # 32*(r%3) + r//3 so runs alternate PE column-groups and up to 3
            # colsum matmuls execute concurrently in the array.
            csum_ps = psumC.tile([128, 512], f32, name="psC")
            for g in range(3):
                nc.tensor.matmul(
                    csum_ps[32 * g:32 * g + 4, :], lhsT=zsel_sb, rhs=ones_sb,
                    start=True, stop=False, skip_group_check=True,
                    tile_position=(0, 32 * g),
                )

            def rhs_pieces(c0, w):
                """Split rotated column range [c0, c0+w) at tile seams."""
                out = []
                for t0, t1, t in ht_tiles:
                    if c0 < t1 and c0 + w > t0:
                        a, b = max(c0, t0), min(c0 + w, t1)
                        out.append((t[:, :, a - t0:b - t0], b - a))
                assert sum(pw for _, pw in out) == w
                return out

            def sim_chunk(ps, pofs, c0, w, start=True):
                """DR matmuls computing rotated cols [c0, c0+w) into ps[:, pofs:]."""
                pieces = rhs_pieces(c0, w)
                for i, (rhs, pw) in enumerate(pieces):
                    if DBG_NO_DR:
                        for k in range(2):
                            nc.tensor.matmul(
                                ps[:, pofs:pofs + pw],
                                lhsT=lhsT_r[:, k, :],
                                rhs=rhs[:, k, :],
                                start=start and (i == 0) and k == 0,
                                stop=(i == len(pieces) - 1) and k == 1,
                                skip_group_check=not start,
                            )
                    else:
                        nc.tensor.matmul(
                            ps[:, pofs:pofs + pw],
                            lhsT=lhsT_r,
                            rhs=rhs,
                            start=start and (i == 0),
                            stop=(i == len(pieces) - 1),
                            perf_mode=DR,
                            skip_group_check=not start,
                        )
                    pofs += pw

            def emit_colsums(r, exp_r, last, qlo=None, qhi=None):
                """PE column sums of exp_r's d=1..31 region into csum_ps.

                q-run p = q//4 lands on partition 32*(p%3) + p//3 at cols
                (q%4)*128; runs cycle the 3 PE column-groups.
                """
                qlo = r if qlo is None else qlo
                qhi = r + 30 if qhi is None else qhi
                for p in range(qlo // 4, qhi // 4 + 1):
                    q0 = max(4 * p, qlo)
                    q1 = min(4 * p + 3, qhi)
                    g, sub = p % 3, p // 3
                    nc.tensor.matmul(
                        csum_ps[32 * g:32 * g + 4,
                                128 * (q0 - 4 * p):128 * (q1 + 1 - 4 * p)],
                        lhsT=csel_sb[:, sub, :],
                        rhs=exp_r[:, 128 * (q0 + 1 - r):128 * (q1 + 2 - r)],
                        start=False,
                        stop=last and p == qhi // 4,
                        skip_group_check=True,
                        tile_position=(0, 32 * g),
                    )

            prev = None  # (r, exp_r)
            for r in range(NRB):
                base = 128 * r
                lhsT_r = rhs_pieces(base, 128)[0][0]
                exp_r = exppool.tile([128, SIMW], bf16, name="exp")

                def dve_sim(j, w):
                    psb = psumB.tile([128, 512], f32, name="psB")
                    sim_chunk(psb, 0, base + ACTW + 512 * j, w)
                    return psb

                def dve_ts(j, w, psb):
                    ti = tipool.tile([128, 512], u16, name="ti")
                    nc.vector.tensor_scalar(
                        ti[:, 0:w], psb[:, 0:w],
                        2.0 * EXP_A16, EXP_B16 - EXP_A16 * M_DEFAULT,
                        ALU.mult, ALU.add,
                    )
                    return ti

                def dve_copy(j, w, ti):
                    nc.vector.tensor_scalar_add(
                        exp_r[:, ACTW + 512 * j:ACTW + 512 * j + w],
                        ti[:, 0:w].bitcast(bf16), 0.0,
                    )

                def dve_red(j, w, ti):
                    nc.vector.reduce_sum(
                        res_sb[:, r * NSLOT + 2 + j:r * NSLOT + 3 + j],
                        ti[:, 0:w].bitcast(bf16),
                        axis=mybir.AxisListType.X,
                    )

                # Diagonal mask first (eye stationary), then sims share one
                # h stationary; the DVE-bank chunks go early so the vector
                # pipeline starts while the PE streams the ACT-bank chunks.
                psA0 = psumA.tile([128, 1536], f32, name="psA")
                nc.tensor.matmul(
                    psA0[:, 0:128], lhsT=eye_sb, rhs=negd_sb,
                    start=True, stop=False, perf_mode=DR,
                )
                if r == 0:
                    # DVE-bank columns arrive last on the DMA pipeline;
                    # stream the ACT banks first.
                    sim_chunk(psA0, 0, base, 512, start=False)
                    sim_chunk(psA0, 512, base + 512, 512)
                    sim_chunk(psA0, 1024, base + 1024, 512)
                    psA1 = psumA.tile([128, 1536], f32, name="psA")
                    for j in range(3):
                        sim_chunk(psA1, 512 * j, base + 1536 + 512 * j, 512)
                    psb0 = dve_sim(0, 512)
                    psb1 = dve_sim(1, 512)
                else:
                    sim_chunk(psA0, 0, base, 512, start=False)
                    psb0 = dve_sim(0, 512)
                    sim_chunk(psA0, 512, base + 512, 512)
                    sim_chunk(psA0, 1024, base + 1024, 512)
                    psb1 = dve_sim(1, 512)
                    psA1 = psumA.tile([128, 1536], f32, name="psA")
                    for j in range(3):
                        sim_chunk(psA1, 512 * j, base + 1536 + 512 * j, 512)

                ti0 = dve_ts(0, 512, psb0)
                ti1 = dve_ts(1, 512, psb1)
                dve_copy(0, 512, ti0)
                dve_copy(1, 512, ti1)

                nc.scalar.activation(
                    out=exp_r[:, 0:1536], in_=psA0,
                    func=mybir.ActivationFunctionType.Exp,
                    bias=biasm, scale=2.0,
                    accum_out=res_sb[:, r * NSLOT:r * NSLOT + 1],
                )

                psb2 = dve_sim(2, 128)
                ti2 = dve_ts(2, 128, psb2)
                dve_copy(2, 128, ti2)
                dve_red(0, 512, ti0)
                dve_red(1, 512, ti1)
                dve_red(2, 128, ti2)

                nc.scalar.activation(
                    out=exp_r[:, 1536:3072], in_=psA1,
                    func=mybir.ActivationFunctionType.Exp,
                    bias=biasm, scale=2.0,
                    accum_out=res_sb[:, r * NSLOT + 1:r * NSLOT + 2],
                )

                if prev is not None and not DBG_NO_COLSUM:
                    emit_colsums(prev[0], prev[1], last=False)
                if r == NRB - 1 and not DBG_NO_COLSUM:
                    emit_colsums(r, exp_r, last=False, qhi=r + 22)

                if r == 6:
                    # Ship finished row-block partials early.
                    nc.sync.dma_start(
                        out=res_dram[:, 0:5 * NSLOT], in_=res_sb[:, 0:5 * NSLOT]
                    )
                prev = (r, exp_r)

            if not DBG_NO_COLSUM:
                emit_colsums(prev[0], prev[1], last=True, qlo=prev[0] + 23)

            # Evacuate colsums PSUM -> SBUF -> DRAM; ship remaining res.
            nc.vector.tensor_scalar_add(csum_sb, csum_ps[0:68, :], 0.0)
            nc.scalar.dma_start(out=csum_dram, in_=csum_sb)
            nc.sync.dma_start(
                out=res_dram[:, 5 * NSLOT:], in_=res_sb[:, 5 * NSLOT:]
            )

    nc.compile()
    _cache["nc"] = nc
    return nc


def _make_inputs(h_i, h_j):
    h = np.concatenate([np.asarray(h_i), np.asarray(h_j)], axis=0).astype(np.float32)
    h8 = np.clip(h, -240.0, 240.0).astype(ml_dtypes.float8_e4m3)
    hT8 = np.ascontiguousarray(h8.T)  # [256, 8192]
    p = np.arange(128)
    head = np.zeros((128, 2, 256), dtype=ml_dtypes.float8_e4m3)
    head[p, :, p] = 1.0          # identity plane (DoubleRow: both k halves)
    head[p, :, 128 + p] = -240.0  # -480*I after the DR pair-sum
    hts = []
    for c in range(NCORES):
        rot = np.roll(hT8, -RPC * c, axis=1)[:, :RHSW]     # [256, 5120]
        rot = rot.reshape(2, 128, RHSW).transpose(1, 0, 2)
        hts.append(np.ascontiguousarray(np.concatenate([head, rot], axis=2)))
    csel = np.zeros((128, 4, 4), dtype=ml_dtypes.bfloat16)
    for q in range(4):
        csel[:, q, q] = 1.0
    return h, hts, csel


def _axon_reset():
    try:
        import ctypes

        lib = ctypes.CDLL("/opt/axon/libaxon_pjrt.so")
        lib.axon_reset.restype = ctypes.c_int64
        return lib.axon_reset() == 0
    except Exception:
        return False


def _run(nc, hts, csel):
    global LAST_RESULTS
    from concourse import bass_utils

    in_maps = [{"ht": hts[c], "csel": csel} for c in range(NCORES)]
    try:
        results = bass_utils.run_bass_kernel_spmd(
            nc, in_maps, core_ids=list(range(NCORES)), trace=TRACE
        )
    except Exception:
        if not _axon_reset():
            raise
        results = bass_utils.run_bass_kernel_spmd(
            nc, in_maps, core_ids=list(range(NCORES)), trace=TRACE
        )
    LAST_RESULTS = results
    return results.results


def kernel(h_i, h_j):
    nc = _build()
    h, hts, csel = _make_inputs(h_i, h_j)
    res = _run(nc, hts, csel)

    S = np.zeros(N, dtype=np.float64)
    for c in range(NCORES):
        r = res[c]["res"].astype(np.float64)          # [128, 48]
        part = r.reshape(128, NRB, NSLOT).sum(axis=2)  # [128, NRB]
        for rb in range(NRB):
            S[RPC * c + 128 * rb:RPC * c + 128 * (rb + 1)] += part[:, rb]
        cs = res[c]["csum"].astype(np.float64)         # [68, 512]
        for q in range(38):
            g = (128 * (q + 1) + RPC * c) % N
            p = q // 4
            S[g:g + 128] += cs[32 * (p % 3) + p // 3,
                               (q % 4) * 128:(q % 4) * 128 + 128]

    # pos on host, exact (f64)
    h_i64 = np.asarray(h_i, dtype=np.float64)
    h_j64 = np.asarray(h_j, dtype=np.float64)
    pos = 2.0 * (h_i64 * h_j64).sum(axis=1)
    pos_sum = 2.0 * pos.sum()

    bad = ~np.isfinite(S) | (S <= 0.0)
    lse = np.where(bad, 0.0, M_DEFAULT + np.log(np.where(bad, 1.0, S)))
    if bad.any():
        # exact host fallback for pathological rows
        h64 = np.concatenate([h_i64, h_j64], axis=0)
        for i in np.nonzero(bad)[0]:
            srow = 2.0 * (h64 @ h64[i])
            srow[i] = -np.inf
            m = srow.max()
            lse[i] = m + np.log(np.exp(srow - m).sum())

    loss = (lse.sum() - pos_sum) / float(N)
    return np.array(loss, dtype=np.float32)


if __name__ == "__main__":
    rng = np.random.default_rng(0)
    h_i = rng.standard_normal((B, D), dtype=np.float32)
    h_j = rng.standard_normal((B, D), dtype=np.float32)
    print("loss:", kernel(h_i, h_j))


# revision 5
# speedup vs baseline: 1.0554x; 1.0554x over previous
"""NT-Xent contrastive loss (forward) on 8 TRN2 NeuronCores via Bass/Tile.

v2: fp8(e4m3) DoubleRow matmuls + symmetric (half-matrix) evaluation.

Math: with h = concat(h_i, h_j) [N=8192, D=256], sim = (h @ h.T) / 0.5,
loss = mean_r( logsumexp_j(sim[r, j], j != r) - pos_r ).  pos is computed
exactly on the host (f64); the device produces per-row partial sums of
exp(sim - M) with a single global shift M = 161.

Symmetry: exp(sim[i,j] - M) == exp(sim[j,i] - M) for a global M, so each
128-row block only computes the cyclic column distances d = 0..32 (of 64
128-col blocks).  Row-sums over d = 0..32 come from ACT (exp + accum_out)
and DVE (Schraudolph bit-trick exp + fused tensor_tensor_reduce); the
remaining distances d = 33..63 for every row are recovered as COLUMN sums
of the d = 1..31 exp tiles, computed by the PE as ones-weighted matmuls
accumulating into a single PSUM bank laid out as [10, 512] f32.

Sharding: core c owns rows [1024c, 1024c + 1024).  Each core receives h.T
column-rotated by its row offset in fp8, pre-transposed to the DoubleRow
layout [128, 2, 5120] (only rotated columns [0, 5120) are ever touched:
row-block r uses columns [128r, 128r + 4224)).  One SPMD program serves
all 8 cores; the host reassembles row sums + column sums in f64, takes
logs, and averages.  Rows whose device sum is non-finite (never, for the
reference data) are recomputed exactly on the host.
"""

import os

import numpy as np
import ml_dtypes

DBG_NO_DR = os.environ.get("DBG_NO_DR", "") != ""
DBG_NO_COLSUM = os.environ.get("DBG_NO_COLSUM", "") != ""
DBG_NO_TTR = os.environ.get("DBG_NO_TTR", "") != ""

B = 4096
D = 256
N = 2 * B
NCORES = 8
RPC = N // NCORES     # 1024 rows per core
NRB = RPC // 128      # 8 row-blocks of 128 per core
M_DEFAULT = 161.0     # global logsumexp shift (rowmax(2 h.h) in [102, 240])
MASK_NEG = -1.0e9

SIMW = 33 * 128       # 4224: sim columns per row-block (distances 0..32)
CSW = 31 * 128        # 3968: colsum columns (distances 1..31)
RHSW = 7 * 128 + SIMW # 5120: rotated columns a core ever reads
HSPLIT = 4352         # first h.T tile: covers row-blocks 0 and 1 entirely
ACTW = 3072           # columns [0, 3072) of each row-block go to ACT
NSLOT = 6             # res slots per row-block (2 ACT + 3 DVE, 1 spare)

# Schraudolph fast-exp in bf16-bit space: exp(y) ~= bitcast_bf16(u16(A*y+B)).
EXP_A16 = float(2 ** 7 / np.log(2.0))
EXP_B16 = 16248.55

TRACE = False
LAST_RESULTS = None

_cache = {}


def _build():
    if "nc" in _cache:
        return _cache["nc"]

    import concourse.tile as tile
    import concourse.mybir as mybir
    from concourse import bacc

    f32 = mybir.dt.float32
    bf16 = mybir.dt.bfloat16
    f8 = mybir.dt.float8e4
    u16 = mybir.dt.uint16
    DR = mybir.MatmulPerfMode.DoubleRow
    ALU = mybir.AluOpType

    nc = bacc.Bacc("TRN2", target_bir_lowering=False, num_devices=NCORES)
    # ht's first 256 columns hold the DoubleRow identity and -240*I mask
    # planes so row 0's first matmuls ride the first DMA chunk.
    ht_dram = nc.dram_tensor("ht", [128, 2, 256 + RHSW], f8, kind="ExternalInput").ap()
    res_dram = nc.dram_tensor("res", [128, NRB * NSLOT], f32, kind="ExternalOutput").ap()
    csel_dram = nc.dram_tensor("csel", [128, 4, 4], bf16, kind="ExternalInput").ap()
    csum_dram = nc.dram_tensor("csum", [68, 512], f32, kind="ExternalOutput").ap()

    with tile.TileContext(nc) as tc:
        with (
            tc.tile_pool(name="hpool", bufs=1) as hpool,
            tc.tile_pool(name="small", bufs=1) as small,
            tc.tile_pool(name="exppool", bufs=2) as exppool,
            tc.tile_pool(name="tipool", bufs=6) as tipool,
            tc.tile_pool(name="psumA", bufs=2, space="PSUM") as psumA,
            tc.tile_pool(name="psumB", bufs=1, space="PSUM") as psumB,
            tc.tile_pool(name="psumC", bufs=1, space="PSUM") as psumC,
        ):
            # --- constants / warmup (overlap the big DMAs) ---

            ones_sb = small.tile([128, 512], bf16)
            nc.vector.memset(ones_sb, 1.0)
            zsel_sb = small.tile([128, 4], bf16)
            nc.vector.memset(zsel_sb, 0.0)
            csel_sb = small.tile([128, 4, 4], bf16)
            nc.gpsimd.dma_start(out=csel_sb, in_=csel_dram)
            wsrc = small.tile([128, 128], bf16)
            nc.vector.memset(wsrc, 0.0)
            res_sb = small.tile([128, NRB * NSLOT], f32)
            nc.vector.memset(res_sb, 0.0)
            csum_sb = small.tile([68, 512], f32)

            # ACT exp-table warm (one tiny activation loads the table).
            warm32 = small.tile([128, 1], f32)
            nc.vector.memset(warm32, 0.0)
            biasm = small.tile([128, 1], f32)
            nc.vector.memset(biasm, -M_DEFAULT)
            nc.scalar.activation(
                out=warm32, in_=warm32,
                func=mybir.ActivationFunctionType.Exp, bias=0.0, scale=0.0,
            )

            # h.T in fp8 DoubleRow layout, split so compute starts on the
            # first piece while the rest streams in.
            ht_tiles = []   # (rotated c0, c1, tile) -- tile col 0 = rot c0
            bounds = [-256, 512, 1536, 3072, RHSW]
            queues = [nc.sync, nc.scalar, nc.sync, nc.gpsimd]
            for bi in range(len(bounds) - 1):
                c0, c1 = bounds[bi], bounds[bi + 1]
                t = hpool.tile([128, 2, c1 - c0], f8, name=f"ht{c0}")
                queues[bi].dma_start(out=t, in_=ht_dram[:, :, c0 + 256:c1 + 256])
                ht_tiles.append((c0, c1, t))
            eye_sb = ht_tiles[0][2][:, :, 0:128]
            negd_sb = ht_tiles[0][2][:, :, 128:256]

            # PE HAM warm: ~32 dummy matmuls while the h.T DMA flies.
            wps = psumA.tile([128, 1536], f32, name="psA")
            for w in range(26):
                nc.tensor.matmul(
                    wps[:, (w % 3) * 512:(w % 3) * 512 + 128],
                    lhsT=wsrc, rhs=wsrc, start=True, stop=True,
                )

            # colsum accumulator: one PSUM bank; q-run r lands on partition
            # 32*(r%3) + r//3 so runs alternate PE column-groups and up to 3
            # colsum matmuls execute concurrently in the array.
            csum_ps = psumC.tile([128, 512], f32, name="psC")
            for g in range(3):
                nc.tensor.matmul(
                    csum_ps[32 * g:32 * g + 4, :], lhsT=zsel_sb, rhs=ones_sb,
                    start=True, stop=False, skip_group_check=True,
                    tile_position=(0, 32 * g),
                )

            def rhs_pieces(c0, w):
                """Split rotated column range [c0, c0+w) at tile seams."""
                out = []
                for t0, t1, t in ht_tiles:
                    if c0 < t1 and c0 + w > t0:
                        a, b = max(c0, t0), min(c0 + w, t1)
                        out.append((t[:, :, a - t0:b - t0], b - a))
                assert sum(pw for _, pw in out) == w
                return out

            def sim_chunk(ps, pofs, c0, w, start=True):
                """DR matmuls computing rotated cols [c0, c0+w) into ps[:, pofs:]."""
                pieces = rhs_pieces(c0, w)
                for i, (rhs, pw) in enumerate(pieces):
                    if DBG_NO_DR:
                        for k in range(2):
                            nc.tensor.matmul(
                                ps[:, pofs:pofs + pw],
                                lhsT=lhsT_r[:, k, :],
                                rhs=rhs[:, k, :],
                                start=start and (i == 0) and k == 0,
                                stop=(i == len(pieces) - 1) and k == 1,
                                skip_group_check=not start,
                            )
                    else:
                        nc.tensor.matmul(
                            ps[:, pofs:pofs + pw],
                            lhsT=lhsT_r,
                            rhs=rhs,
                            start=start and (i == 0),
                            stop=(i == len(pieces) - 1),
                            perf_mode=DR,
                            skip_group_check=not start,
                        )
                    pofs += pw

            def emit_colsums(r, exp_r, last, qlo=None, qhi=None):
                """PE column sums of exp_r's d=1..31 region into csum_ps.

                q-run p = q//4 lands on partition 32*(p%3) + p//3 at cols
                (q%4)*128; runs cycle the 3 PE column-groups.
                """
                qlo = r if qlo is None else qlo
                qhi = r + 30 if qhi is None else qhi
                for p in range(qlo // 4, qhi // 4 + 1):
                    q0 = max(4 * p, qlo)
                    q1 = min(4 * p + 3, qhi)
                    g, sub = p % 3, p // 3
                    nc.tensor.matmul(
                        csum_ps[32 * g:32 * g + 4,
                                128 * (q0 - 4 * p):128 * (q1 + 1 - 4 * p)],
                        lhsT=csel_sb[:, sub, :],
                        rhs=exp_r[:, 128 * (q0 + 1 - r):128 * (q1 + 2 - r)],
                        start=False,
                        stop=last and p == qhi // 4,
                        skip_group_check=True,
                        tile_position=(0, 32 * g),
                    )

            prev = None  # (r, exp_r)
            pending_reds = []
            for r in range(NRB):
                base = 128 * r
                lhsT_r = rhs_pieces(base, 128)[0][0]
                exp_r = exppool.tile([128, SIMW], bf16, name="exp")

                def dve_sim(j, w):
                    psb = psumB.tile([128, 512], f32, name="psB")
                    sim_chunk(psb, 0, base + ACTW + 512 * j, w)
                    return psb

                def dve_ts(j, w, psb):
                    ti = tipool.tile([128, 512], u16, name="ti")
                    nc.vector.tensor_scalar(
                        ti[:, 0:w], psb[:, 0:w],
                        2.0 * EXP_A16, EXP_B16 - EXP_A16 * M_DEFAULT,
                        ALU.mult, ALU.add,
                    )
                    return ti

                def dve_copy(j, w, ti):
                    nc.vector.tensor_scalar_add(
                        exp_r[:, ACTW + 512 * j:ACTW + 512 * j + w],
                        ti[:, 0:w].bitcast(bf16), 0.0,
                    )

                def dve_red(slot, w, ti):
                    nc.vector.reduce_sum(
                        res_sb[:, slot:slot + 1],
                        ti[:, 0:w].bitcast(bf16),
                        axis=mybir.AxisListType.X,
                    )

                # Diagonal mask first (eye stationary), then sims share one
                # h stationary; the DVE-bank chunks go early so the vector
                # pipeline starts while the PE streams the ACT-bank chunks.
                psA0 = psumA.tile([128, 1536], f32, name="psA")
                nc.tensor.matmul(
                    psA0[:, 0:128], lhsT=eye_sb, rhs=negd_sb,
                    start=True, stop=False, perf_mode=DR,
                )
                if r == 0:
                    # DVE-bank columns arrive last on the DMA pipeline;
                    # stream the ACT banks first.
                    sim_chunk(psA0, 0, base, 512, start=False)
                    sim_chunk(psA0, 512, base + 512, 512)
                    sim_chunk(psA0, 1024, base + 1024, 512)
                    psA1 = psumA.tile([128, 1536], f32, name="psA")
                    for j in range(3):
                        sim_chunk(psA1, 512 * j, base + 1536 + 512 * j, 512)
                    psb0 = dve_sim(0, 512)
                    psb1 = dve_sim(1, 512)
                else:
                    sim_chunk(psA0, 0, base, 512, start=False)
                    psb0 = dve_sim(0, 512)
                    sim_chunk(psA0, 512, base + 512, 512)
                    sim_chunk(psA0, 1024, base + 1024, 512)
                    psA1 = psumA.tile([128, 1536], f32, name="psA")
                    sim_chunk(psA1, 0, base + 1536, 512)
                    psb1 = dve_sim(1, 512)
                    sim_chunk(psA1, 512, base + 2048, 512)
                    sim_chunk(psA1, 1024, base + 2560, 512)

                ti0 = dve_ts(0, 512, psb0)
                ti1 = dve_ts(1, 512, psb1)
                for red in pending_reds:
                    red()
                pending_reds = []
                dve_copy(0, 512, ti0)
                dve_copy(1, 512, ti1)

                nc.scalar.activation(
                    out=exp_r[:, 0:1536], in_=psA0,
                    func=mybir.ActivationFunctionType.Exp,
                    bias=biasm, scale=2.0,
                    accum_out=res_sb[:, r * NSLOT:r * NSLOT + 1],
                )

                psb2 = dve_sim(2, 128)
                ti2 = dve_ts(2, 128, psb2)
                dve_copy(2, 128, ti2)
                pending_reds = [
                    (lambda f=dve_red, s_=r * NSLOT + 2 + j, w=w, ti=ti: f(s_, w, ti))
                    for j, w, ti in ((0, 512, ti0), (1, 512, ti1), (2, 128, ti2))
                ]

                nc.scalar.activation(
                    out=exp_r[:, 1536:3072], in_=psA1,
                    func=mybir.ActivationFunctionType.Exp,
                    bias=biasm, scale=2.0,
                    accum_out=res_sb[:, r * NSLOT + 1:r * NSLOT + 2],
                )

                if prev is not None and not DBG_NO_COLSUM:
                    emit_colsums(prev[0], prev[1], last=False)
                if r == NRB - 1 and not DBG_NO_COLSUM:
                    emit_colsums(r, exp_r, last=False, qhi=r + 22)

                if r == 6:
                    # Ship finished row-block partials early.
                    nc.sync.dma_start(
                        out=res_dram[:, 0:5 * NSLOT], in_=res_sb[:, 0:5 * NSLOT]
                    )
                prev = (r, exp_r)

            for red in pending_reds:
                red()
            if not DBG_NO_COLSUM:
                emit_colsums(prev[0], prev[1], last=True, qlo=prev[0] + 23)

            # Evacuate colsums PSUM -> SBUF -> DRAM; ship remaining res.
            nc.vector.tensor_scalar_add(csum_sb, csum_ps[0:68, :], 0.0)
            nc.scalar.dma_start(out=csum_dram, in_=csum_sb)
            nc.sync.dma_start(
                out=res_dram[:, 5 * NSLOT:], in_=res_sb[:, 5 * NSLOT:]
            )

    nc.compile()
    _cache["nc"] = nc
    return nc


def _make_inputs(h_i, h_j):
    h = np.concatenate([np.asarray(h_i), np.asarray(h_j)], axis=0).astype(np.float32)
    h8 = np.clip(h, -240.0, 240.0).astype(ml_dtypes.float8_e4m3)
    hT8 = np.ascontiguousarray(h8.T)  # [256, 8192]
    p = np.arange(128)
    head = np.zeros((128, 2, 256), dtype=ml_dtypes.float8_e4m3)
    head[p, :, p] = 1.0          # identity plane (DoubleRow: both k halves)
    head[p, :, 128 + p] = -240.0  # -480*I after the DR pair-sum
    hts = []
    for c in range(NCORES):
        rot = np.roll(hT8, -RPC * c, axis=1)[:, :RHSW]     # [256, 5120]
        rot = rot.reshape(2, 128, RHSW).transpose(1, 0, 2)
        hts.append(np.ascontiguousarray(np.concatenate([head, rot], axis=2)))
    csel = np.zeros((128, 4, 4), dtype=ml_dtypes.bfloat16)
    for q in range(4):
        csel[:, q, q] = 1.0
    return h, hts, csel


def _axon_reset():
    try:
        import ctypes

        lib = ctypes.CDLL("/opt/axon/libaxon_pjrt.so")
        lib.axon_reset.restype = ctypes.c_int64
        return lib.axon_reset() == 0
    except Exception:
        return False


def _run(nc, hts, csel):
    global LAST_RESULTS
    from concourse import bass_utils

    in_maps = [{"ht": hts[c], "csel": csel} for c in range(NCORES)]
    try:
        results = bass_utils.run_bass_kernel_spmd(
            nc, in_maps, core_ids=list(range(NCORES)), trace=TRACE
        )
    except Exception:
        if not _axon_reset():
            raise
        results = bass_utils.run_bass_kernel_spmd(
            nc, in_maps, core_ids=list(range(NCORES)), trace=TRACE
        )
    LAST_RESULTS = results
    return results.results


def kernel(h_i, h_j):
    nc = _build()
    h, hts, csel = _make_inputs(h_i, h_j)
    res = _run(nc, hts, csel)

    S = np.zeros(N, dtype=np.float64)
    for c in range(NCORES):
        r = res[c]["res"].astype(np.float64)          # [128, 48]
        part = r.reshape(128, NRB, NSLOT).sum(axis=2)  # [128, NRB]
        for rb in range(NRB):
            S[RPC * c + 128 * rb:RPC * c + 128 * (rb + 1)] += part[:, rb]
        cs = res[c]["csum"].astype(np.float64)         # [68, 512]
        for q in range(38):
            g = (128 * (q + 1) + RPC * c) % N
            p = q // 4
            S[g:g + 128] += cs[32 * (p % 3) + p // 3,
                               (q % 4) * 128:(q % 4) * 128 + 128]

    # pos on host, exact (f64)
    h_i64 = np.asarray(h_i, dtype=np.float64)
    h_j64 = np.asarray(h_j, dtype=np.float64)
    pos = 2.0 * (h_i64 * h_j64).sum(axis=1)
    pos_sum = 2.0 * pos.sum()

    bad = ~np.isfinite(S) | (S <= 0.0)
    lse = np.where(bad, 0.0, M_DEFAULT + np.log(np.where(bad, 1.0, S)))
    if bad.any():
        # exact host fallback for pathological rows
        h64 = np.concatenate([h_i64, h_j64], axis=0)
        for i in np.nonzero(bad)[0]:
            srow = 2.0 * (h64 @ h64[i])
            srow[i] = -np.inf
            m = srow.max()
            lse[i] = m + np.log(np.exp(srow - m).sum())

    loss = (lse.sum() - pos_sum) / float(N)
    return np.array(loss, dtype=np.float32)


if __name__ == "__main__":
    rng = np.random.default_rng(0)
    h_i = rng.standard_normal((B, D), dtype=np.float32)
    h_j = rng.standard_normal((B, D), dtype=np.float32)
    print("loss:", kernel(h_i, h_j))
